# revision 1
# baseline (speedup 1.0000x reference)
"""MoE-Mamba block kernel for 8 Trainium2 NeuronCores — round 1 perf rework.

Sharding: core c = (batch b = c//2, d_inner half = c%2). Each core computes
the full in_proj xc columns (2048, channel-permuted so its own half comes
first) plus its z half, the causal depthwise conv + SiLU, x_proj (needs full
xc), dt_proj/softplus for its half, the selective scan over its 1024
channels x 16 states, gating, and its out_proj partial. Pair-wise
ReduceScatters (split in two for overlap) sum the out_proj partials and hand
each core two L-quarters, on which it does residual + LayerNorm + LeakyReLU.

Round-1 changes vs baseline:
- GP tensor_tensor offload removed: concurrent GP SBUF streaming derated
  DVE TTs 3.6x (692ns -> 2500ns measured); all scan elementwise now on DVE.
- B/C partition-broadcasts hoisted before the z-half in_proj so GP overlaps
  PE/ACT instead of stalling the scan start (~60us gap).
- Matmuls widened to FD=1024 (half the instruction count + drains).
- x/W_in loads interleaved per k-tile so the first matmul starts earlier.
- ReduceScatter split in two; out_proj half B and LN half A overlap the
  collectives. gamma/beta broadcasts hoisted to kernel start.
- LayerNorm Newton iteration batched across row-tiles ([128,2] per half).
"""

import os
import sys

import numpy as np

try:
    import ml_dtypes
except ImportError:  # pragma: no cover
    ml_dtypes = None


def _ensure_import():
    try:
        import concourse  # noqa: F401
    except ImportError:
        for p in ("/opt/trn_rl_repo", os.path.expanduser("~/.axon_site/_ro/trn_rl_repo")):
            if os.path.isdir(p):
                sys.path.insert(0, p)
                break


_ensure_import()
os.environ.setdefault("MYCRO_LOCAL_CACHE", "1")

from contextlib import ExitStack  # noqa: E402

import concourse.bass as bass  # noqa: E402
import concourse.tile as tile  # noqa: E402
from concourse import bacc, mybir  # noqa: E402

F32 = mybir.dt.float32
BF16 = mybir.dt.bfloat16
FP8 = mybir.dt.float8e4
AF = mybir.ActivationFunctionType
OP = mybir.AluOpType

D_MODEL = 1024
D_INNER = 2048
D_STATE = 16
D_CONV = 4
DT_RANK = 64
BATCH = 4
SEQ = 1024
DH = D_INNER // 2  # channels per core
P = 128
KT = D_MODEL // P          # 8  k-tiles over d_model
MT_XC = D_INNER // P       # 16 m-tiles of xc
MT_Z = DH // P             # 8  m-tiles of z
MH = DH // P               # 8  d-tiles per core in the scan
LQ = SEQ // 4              # 256 rows per collective-half per core
LN_EPS = 1e-5
LRELU = 0.01
WSCALE = 16.0              # exact power-of-two rescale for fp8 in_proj weights

N_CORES = 8
REPLICA_GROUPS = [[0, 1], [2, 3], [4, 5], [6, 7]]


def build_program(a_n, enable_asserts=False, sim_safe=False):
    """Build + compile the single-core SPMD Bass program. a_n: 16 floats."""
    nc = bacc.Bacc(
        "TRN2",
        target_bir_lowering=False,
        debug=False,
        enable_asserts=enable_asserts,
        num_devices=N_CORES,
    )

    # ---- I/O declarations (per-core shards; names match _shard_inputs) ----
    xt_d = nc.dram_tensor("xt", [D_MODEL, SEQ], FP8, kind="ExternalInput").ap()
    xres_d = nc.dram_tensor("xres", [2 * LQ, D_MODEL], F32, kind="ExternalInput").ap()
    win_d = nc.dram_tensor("win", [D_MODEL, D_INNER + DH], FP8, kind="ExternalInput").ap()
    cw_d = nc.dram_tensor("convw", [P, MT_XC * D_CONV], F32, kind="ExternalInput").ap()
    cb_d = nc.dram_tensor("convb", [P, MT_XC], F32, kind="ExternalInput").ap()
    wx_d = nc.dram_tensor("wx", [D_INNER, DT_RANK + 2 * D_STATE], BF16, kind="ExternalInput").ap()
    wdt_d = nc.dram_tensor("wdt", [DT_RANK, DH], BF16, kind="ExternalInput").ap()
    bdt_d = nc.dram_tensor("bdt", [P, MH], F32, kind="ExternalInput").ap()
    dv_d = nc.dram_tensor("dvec", [P, MH], F32, kind="ExternalInput").ap()
    wout_d = nc.dram_tensor("wout", [DH, D_MODEL], BF16, kind="ExternalInput").ap()
    gamma_d = nc.dram_tensor("gamma", [1, D_MODEL], BF16, kind="ExternalInput").ap()
    beta_d = nc.dram_tensor("beta", [1, D_MODEL], BF16, kind="ExternalInput").ap()
    eye_d = nc.dram_tensor("eye", [P, P], BF16, kind="ExternalInput").ap()
    out_d = nc.dram_tensor("out_half", [2 * LQ, D_MODEL], F32, kind="ExternalOutput").ap()

    NPROJ = DT_RANK + 2 * D_STATE  # 96

    with tile.TileContext(nc) as tc, ExitStack() as es:
        pers = es.enter_context(tc.tile_pool(name="pers", bufs=1))
        ps = es.enter_context(tc.tile_pool(name="psum", bufs=3, space="PSUM"))
        dram = es.enter_context(tc.tile_pool(name="dram", bufs=1, space="DRAM"))

        # ---- small constants ----
        cw_sb = pers.tile([P, MT_XC * D_CONV], F32, name="cw_sb")
        nc.sync.dma_start(cw_sb[:], cw_d[:])
        cb_sb = pers.tile([P, MT_XC], F32, name="cb_sb")
        nc.sync.dma_start(cb_sb[:], cb_d[:])
        bdt_sb = pers.tile([P, MH], F32, name="bdt_sb")
        nc.sync.dma_start(bdt_sb[:], bdt_d[:])
        dv_sb = pers.tile([P, MH], F32, name="dv_sb")
        nc.sync.dma_start(dv_sb[:], dv_d[:])
        eye16 = pers.tile([P, P], BF16, name="eye16")
        nc.sync.dma_start(eye16[:], eye_d[:])
        # gamma/beta broadcasts (GP idle here; used at the very end)
        g16 = pers.tile([1, D_MODEL], BF16, name="g16")
        nc.sync.dma_start(g16[:], gamma_d[:])
        b16 = pers.tile([1, D_MODEL], BF16, name="b16")
        nc.sync.dma_start(b16[:], beta_d[:])
        gb_g = pers.tile([P, D_MODEL], BF16, name="gb_g")
        nc.gpsimd.partition_broadcast(gb_g[:], g16[0:1, :])
        gb_b = pers.tile([P, D_MODEL], BF16, name="gb_b")
        nc.gpsimd.partition_broadcast(gb_b[:], b16[0:1, :])

        # ---- medium-lived bf16 tensors ----
        bc16 = pers.tile([2 * D_STATE, SEQ], BF16, name="bc16")
        dtraw16 = pers.tile([DT_RANK, SEQ], BF16, name="dtraw16")
        wx16 = [pers.tile([P, NPROJ], BF16, name=f"wx16_{k}") for k in range(MT_XC)]
        wdt16 = pers.tile([DT_RANK, DH], BF16, name="wdt16")

        p_ug = es.enter_context(tc.tile_pool(name="p_ug", bufs=1))  # until gating
        u16m = [p_ug.tile([P, SEQ], BF16, name=f"u16m_{m}") for m in range(MH)]
        zs16 = [p_ug.tile([P, SEQ], BF16, name=f"zs16_{m}") for m in range(MT_Z)]

        p_y = es.enter_context(tc.tile_pool(name="p_y", bufs=1))
        p_bc = es.enter_context(tc.tile_pool(name="p_bc", bufs=1))

        # =========== stage A: in_proj (fp8) + conv (PE diag) + x_proj ===========
        with (
            tc.tile_pool(name="p_xw", bufs=1) as pxw,
            tc.tile_pool(name="p_ld", bufs=2) as pld,
            tc.tile_pool(name="p_uo", bufs=2) as puo,
        ):
            # inputs arrive pre-cast (fp8/bf16) from the host: DMA directly
            xt8 = []
            w8 = []
            for k in range(KT):
                t = pxw.tile([P, SEQ], FP8, name=f"xt8_{k}")
                nc.sync.dma_start(t[:], xt_d[k * P : (k + 1) * P, :])
                xt8.append(t)
                w = pxw.tile([P, D_INNER + DH], FP8, name=f"w8_{k}")
                nc.sync.dma_start(w[:], win_d[k * P : (k + 1) * P, :])
                w8.append(w)
            for k in range(MT_XC):
                nc.sync.dma_start(wx16[k][:], wx_d[k * P : (k + 1) * P, :])
            nc.sync.dma_start(wdt16[:], wdt_d[:])

            px = [ps.tile([NPROJ, SEQ // 2], F32, tag="xp", bufs=2, name=f"px_{i}")
                  for i in range(2)]

            def in_proj_tile(mt):
                """One [128, SEQ] column tile of x @ W_in -> PSUM pair (fp8)."""
                pts = [ps.tile([P, SEQ // 2], F32, tag="mm", bufs=4, name=f"pt_{mt}_{i}")
                       for i in range(2)]
                for k in range(KT):
                    for lhv in range(2):
                        nc.tensor.matmul(
                            pts[lhv][:],
                            w8[k][:, mt * P : (mt + 1) * P],
                            xt8[k][:, lhv * 512 : (lhv + 1) * 512],
                            start=(k == 0),
                            stop=(k == KT - 1),
                        )
                return pts

            # --- xc half: in_proj -> conv -> silu -> x_proj (accumulated) ---
            # xcp/diag tiles live only for this loop (freed before broadcasts)
            es_a = ExitStack()
            pconv = es_a.enter_context(tc.tile_pool(name="p_conv", bufs=1))
            pdg = es_a.enter_context(tc.tile_pool(name="p_dg", bufs=8))
            xcp = [pconv.tile([P, SEQ + 4], BF16, name=f"xcp_{m}") for m in range(MT_XC)]
            for mt in range(MT_XC):
                dst = xcp[mt]
                nc.vector.memset(dst[:, 0:4], 0.0)
                pts = in_proj_tile(mt)
                for lhv in range(2):
                    nc.scalar.activation(
                        dst[:, 4 + lhv * 512 : 4 + (lhv + 1) * 512], pts[lhv][:],
                        AF.Copy, scale=1.0 / WSCALE,
                    )
                # conv u = silu(sum_j w_j xc[l-3+j] + b) as 4 accumulating
                # diagonal matmuls: lhsT = diag(w_j), rhs = shifted xcp
                diags = []
                for j in range(D_CONV):
                    dg = pdg.tile([P, P], BF16, tag="dg")
                    nc.vector.tensor_scalar(
                        dg[:], eye16[:], cw_sb[:, mt * 4 + j : mt * 4 + j + 1],
                        None, op0=OP.mult,
                    )
                    diags.append(dg)
                pus = [ps.tile([P, SEQ // 2], F32, tag="mm", bufs=4, name=f"pu_{mt}_{i}")
                       for i in range(2)]
                for j in range(D_CONV):
                    for lhv in range(2):
                        nc.tensor.matmul(
                            pus[lhv][:],
                            diags[j][:],
                            xcp[mt][:, 1 + j + lhv * 512 : 1 + j + lhv * 512 + 512],
                            start=(j == 0),
                            stop=(j == D_CONV - 1),
                        )
                ut = u16m[mt] if mt < MH else puo.tile([P, SEQ], BF16, tag="uo")
                for lhv in range(2):
                    usl = ut[:, lhv * 512 : (lhv + 1) * 512]
                    pu = pus[lhv]
                    if not sim_safe:
                        nc.scalar.activation(
                            usl, pu[:], AF.Silu, bias=cb_sb[:, mt : mt + 1], scale=1.0
                        )
                    else:
                        ub = puo.tile([P, 512], BF16, tag="ub")
                        nc.scalar.activation(ub[:], pu[:], AF.Copy, scale=1.0)
                        nc.vector.tensor_scalar(
                            ub[:], ub[:], cb_sb[:, mt : mt + 1], None, op0=OP.add
                        )
                        nc.scalar.activation(usl, ub[:], AF.Sigmoid)
                        nc.vector.tensor_tensor(usl, usl, ub[:], op=OP.mult)
                # x_proj accumulation for this channel tile
                for lhv in range(2):
                    nc.tensor.matmul(
                        px[lhv][:],
                        wx16[mt][:],
                        ut[:, lhv * 512 : (lhv + 1) * 512],
                        start=(mt == 0),
                        stop=(mt == MT_XC - 1),
                    )

            es_a.close()  # free xcp/diags before the B/C broadcast tiles

            # --- drain x_proj ---
            for lhv in range(2):
                nc.scalar.activation(
                    dtraw16[:, lhv * 512 : (lhv + 1) * 512], px[lhv][0:DT_RANK, :], AF.Copy
                )
                nc.scalar.activation(
                    bc16[:, lhv * 512 : (lhv + 1) * 512], px[lhv][DT_RANK:NPROJ, :], AF.Copy
                )

            # --- B/C broadcasts on GP, overlapping the z-half in_proj below ---
            bb16, cbt16 = [], []
            for n in range(D_STATE):
                brow = puo.tile([1, SEQ], BF16, tag="row")
                nc.sync.dma_start(brow[:], bc16[n : n + 1, :])
                bb = p_bc.tile([P, SEQ], BF16, name=f"bb_{n}")
                nc.gpsimd.partition_broadcast(bb[:], brow[0:1, :])
                crow = puo.tile([1, SEQ], BF16, tag="row")
                nc.sync.dma_start(crow[:], bc16[D_STATE + n : D_STATE + n + 1, :])
                cb_t = p_bc.tile([P, SEQ], BF16, name=f"cbt_{n}")
                nc.gpsimd.partition_broadcast(cb_t[:], crow[0:1, :])
                bb16.append(bb)
                cbt16.append(cb_t)

            # --- z half: in_proj -> silu (PE/ACT overlap the GP broadcasts) ---
            for mz in range(MT_Z):
                pts = in_proj_tile(MT_XC + mz)
                for lhv in range(2):
                    zsl = zs16[mz][:, lhv * 512 : (lhv + 1) * 512]
                    pt = pts[lhv]
                    if not sim_safe:
                        nc.scalar.activation(zsl, pt[:], AF.Silu, scale=1.0 / WSCALE)
                    else:
                        zb = puo.tile([P, 512], BF16, tag="zb")
                        nc.scalar.activation(zb[:], pt[:], AF.Copy, scale=1.0 / WSCALE)
                        nc.scalar.activation(zsl, pt[:], AF.Sigmoid, scale=1.0 / WSCALE)
                        nc.vector.tensor_tensor(zsl, zsl, zb[:], op=OP.mult)

        # =========== stage C: selective scan (m outer, n inner) ===========
        with (
            tc.tile_pool(name="p_dtm", bufs=2) as pdtm,
            tc.tile_pool(name="p_scan", bufs=3) as psc,
        ):
            y16 = [p_y.tile([P, SEQ], BF16, name=f"y16_{m}") for m in range(MH)]

            for m in range(MH):
                # dt_proj + softplus + dtu for this m-tile.
                # softplus(v) = Ln(g + 1) with g = e^v (both on ACT; the
                # natural_log_exp table holds Exp and Ln so no table thrash).
                g = pdtm.tile([P, SEQ], BF16, tag="g")
                for lhv in range(2):
                    pt = ps.tile([P, SEQ // 2], F32, tag="mm", bufs=4)
                    nc.tensor.matmul(
                        pt[:],
                        wdt16[:, m * P : (m + 1) * P],
                        dtraw16[:, lhv * 512 : (lhv + 1) * 512],
                        start=True,
                        stop=True,
                    )
                    nc.scalar.activation(
                        g[:, lhv * 512 : (lhv + 1) * 512], pt[:], AF.Exp,
                        bias=bdt_sb[:, m : m + 1], scale=1.0,
                    )
                dt16m = pdtm.tile([P, SEQ], BF16, tag="dt16m")
                nc.scalar.activation(dt16m[:], g[:], AF.Ln, bias=1.0)
                dtu16m = pdtm.tile([P, SEQ], BF16, tag="dtu16m")
                nc.vector.tensor_tensor(dtu16m[:], dt16m[:], u16m[m][:], op=OP.mult)
                ypsum = [
                    ps.tile([P, SEQ // 2], F32, tag="ymm", bufs=2, name=f"yp_{m}_{i}")
                    for i in range(2)
                ]
                for n in range(D_STATE):
                    da = psc.tile([P, SEQ], BF16, tag="da")
                    nc.scalar.activation(da[:], dt16m[:], AF.Exp, scale=float(a_n[n]))
                    dbu = psc.tile([P, SEQ], BF16, tag="dbu")
                    nc.vector.tensor_tensor(dbu[:], dtu16m[:], bb16[n][:], op=OP.mult)
                    h = psc.tile([P, SEQ], BF16, tag="h")
                    nc.vector.tensor_tensor_scan(h[:], da[:], dbu[:], 0.0, op0=OP.mult, op1=OP.add)
                    yt = psc.tile([P, SEQ], BF16, tag="yt")
                    nc.vector.tensor_tensor(yt[:], h[:], cbt16[n][:], op=OP.mult)
                    # accumulate y in PSUM via identity matmul (PE has slack)
                    for lhv in range(2):
                        nc.tensor.matmul(
                            ypsum[lhv][:],
                            eye16[:],
                            yt[:, lhv * 512 : (lhv + 1) * 512],
                            start=(n == 0),
                            stop=(n == D_STATE - 1),
                        )
                # D-term (+ y from PSUM) then gating
                for lhv in range(2):
                    sl = slice(lhv * 512, (lhv + 1) * 512)
                    nc.vector.scalar_tensor_tensor(
                        y16[m][:, sl], u16m[m][:, sl], dv_sb[:, m : m + 1], ypsum[lhv][:],
                        op0=OP.mult, op1=OP.add,
                    )
                nc.vector.tensor_tensor(y16[m][:], y16[m][:], zs16[m][:], op=OP.mult)

        # =========== stage D: out_proj -> bounce -> 2x ReduceScatter ===========
        binb = [dram.tile([SEQ // 2, D_MODEL], BF16, name=f"bounce_in{i}") for i in range(2)]
        bout = [dram.tile([LQ, D_MODEL], BF16, name=f"bounce_out{i}") for i in range(2)]
        with (
            tc.tile_pool(name="p_wo", bufs=1) as pwo,
            tc.tile_pool(name="p_op", bufs=3) as pop,
            tc.tile_pool(name="p_ln", bufs=2) as pln,
        ):
            wout16 = []
            for k in range(MH):
                t = pwo.tile([P, D_MODEL], BF16, name=f"wout16_{k}")
                nc.sync.dma_start(t[:], wout_d[k * P : (k + 1) * P, :])
                wout16.append(t)

            def out_proj_quarter(half):
                for lt in range(4 * half, 4 * half + 4):
                    op_sb = pop.tile([P, D_MODEL], BF16, tag="op")
                    pts = [ps.tile([P, 512], F32, tag="mm", bufs=4, name=f"po_{lt}_{i}")
                           for i in range(2)]
                    for k in range(MH):
                        for nf in range(2):
                            nc.tensor.matmul(
                                pts[nf][:],
                                y16[k][:, lt * P : (lt + 1) * P],
                                wout16[k][:, nf * 512 : (nf + 1) * 512],
                                start=(k == 0),
                                stop=(k == MH - 1),
                            )
                    for nf in range(2):
                        nc.scalar.activation(
                            op_sb[:, nf * 512 : (nf + 1) * 512], pts[nf][:], AF.Copy
                        )
                    nc.sync.dma_start(
                        binb[half][(lt - 4 * half) * P : (lt - 4 * half + 1) * P, :],
                        op_sb[:],
                    )

            def layer_norm_quarter(half):
                # residual + stats for the two 128-row tiles, batched Newton,
                # then normalize+affine+leakyrelu
                hres_t, v_all, mu_all = [], None, None
                v_all = pln.tile([P, 2], F32, tag="vall")
                mu_all = pln.tile([P, 2], F32, tag="muall")
                for i in range(2):
                    h16 = pln.tile([P, D_MODEL], BF16, tag="h16")
                    nc.sync.dma_start(h16[:], bout[half][i * P : (i + 1) * P, :])
                    xr = pln.tile([P, D_MODEL], F32, tag="xr")
                    nc.sync.dma_start(
                        xr[:], xres_d[(2 * half + i) * P : (2 * half + i + 1) * P, :]
                    )
                    hres = pln.tile([P, D_MODEL], F32, tag=f"hres{i}")
                    nc.vector.tensor_tensor(hres[:], h16[:], xr[:], op=OP.add)
                    hres_t.append(hres)
                    ssum = pln.tile([P, 1], F32, tag="ssum")
                    nc.vector.tensor_reduce(ssum[:], hres[:], axis=mybir.AxisListType.X, op=OP.add)
                    sq = pln.tile([P, D_MODEL], F32, tag="sq")
                    ssq = pln.tile([P, 1], F32, tag="ssq")
                    nc.scalar.activation(sq[:], hres[:], AF.Square, accum_out=ssq[:])
                    nc.vector.tensor_scalar(
                        mu_all[:, i : i + 1], ssum[:], 1.0 / D_MODEL, None, op0=OP.mult
                    )
                    v = pln.tile([P, 1], F32, tag="v")
                    nc.vector.tensor_tensor(
                        v[:], mu_all[:, i : i + 1], mu_all[:, i : i + 1], op=OP.mult
                    )
                    nc.vector.scalar_tensor_tensor(
                        v[:], ssq[:], 1.0 / D_MODEL, v[:], op0=OP.mult, op1=OP.subtract
                    )
                    nc.vector.tensor_scalar(
                        v_all[:, i : i + 1], v[:], LN_EPS, None, op0=OP.add
                    )
                yv = pln.tile([P, 2], F32, tag="yv")
                nc.vector.memset(yv[:], 1.0)
                t = pln.tile([P, 2], F32, tag="t")
                for _ in range(5):
                    nc.vector.tensor_tensor(t[:], yv[:], yv[:], op=OP.mult)
                    nc.vector.tensor_tensor(t[:], t[:], v_all[:], op=OP.mult)
                    nc.vector.tensor_scalar(t[:], t[:], -0.5, 1.5, op0=OP.mult, op1=OP.add)
                    nc.vector.tensor_tensor(yv[:], yv[:], t[:], op=OP.mult)
                nb = pln.tile([P, 2], F32, tag="nb")
                nc.vector.tensor_tensor(nb[:], mu_all[:], yv[:], op=OP.mult)
                nc.vector.tensor_scalar(nb[:], nb[:], -1.0, None, op0=OP.mult)
                for i in range(2):
                    xn = pln.tile([P, D_MODEL], F32, tag="xn")
                    nc.scalar.activation(
                        xn[:], hres_t[i][:], AF.Identity,
                        bias=nb[:, i : i + 1], scale=yv[:, i : i + 1],
                    )
                    nc.vector.tensor_tensor(xn[:], xn[:], gb_g[:], op=OP.mult)
                    nc.vector.tensor_tensor(xn[:], xn[:], gb_b[:], op=OP.add)
                    fin = pln.tile([P, D_MODEL], F32, tag="fin")
                    nc.vector.scalar_tensor_tensor(
                        fin[:], xn[:], LRELU, xn[:], op0=OP.mult, op1=OP.max
                    )
                    nc.sync.dma_start(
                        out_d[(2 * half + i) * P : (2 * half + i + 1) * P, :], fin[:]
                    )

            out_proj_quarter(0)
            nc.gpsimd.collective_compute(
                "ReduceScatter",
                OP.add,
                replica_groups=REPLICA_GROUPS,
                ins=[binb[0].opt()],
                outs=[bout[0].opt()],
            )
            layer_norm_quarter(0)
            out_proj_quarter(1)
            nc.gpsimd.collective_compute(
                "ReduceScatter",
                OP.add,
                replica_groups=REPLICA_GROUPS,
                ins=[binb[1].opt()],
                outs=[bout[1].opt()],
            )
            layer_norm_quarter(1)

    nc.compile()
    return nc


def _shard_inputs(inputs):
    x = np.asarray(inputs["x"], np.float32)
    W_in = np.asarray(inputs["W_in"], np.float32)
    conv_w = np.asarray(inputs["conv_w"], np.float32)
    conv_b = np.asarray(inputs["conv_b"], np.float32)
    W_x = np.asarray(inputs["W_x"], np.float32)
    W_dt = np.asarray(inputs["W_dt"], np.float32)
    b_dt = np.asarray(inputs["b_dt"], np.float32)
    Dp = np.asarray(inputs["D"], np.float32)
    W_out = np.asarray(inputs["W_out"], np.float32)
    gamma = np.asarray(inputs["gamma"], np.float32)
    beta = np.asarray(inputs["beta"], np.float32)

    def col_tiles(v):  # [DH] -> [P, MH] (tile-major columns)
        return np.ascontiguousarray(v.reshape(-1, P).T)

    FP8NP = ml_dtypes.float8_e4m3fn
    BF16NP = ml_dtypes.bfloat16

    in_maps = []
    for c in range(N_CORES):
        b, half = divmod(c, 2)
        perm = np.concatenate(
            [np.arange(half * DH, (half + 1) * DH), np.arange((1 - half) * DH, (2 - half) * DH)]
        )
        cw = conv_w[perm]  # [2048, 4]
        # this core owns t-quarters [half*256, half*256+256) and +512
        q = [slice(half * LQ, (half + 1) * LQ), slice(512 + half * LQ, 512 + (half + 1) * LQ)]
        m = {
            "xt": np.ascontiguousarray(x[b].T).astype(FP8NP),
            "xres": np.ascontiguousarray(np.concatenate([x[b, q[0]], x[b, q[1]]], axis=0)),
            "win": np.ascontiguousarray(
                WSCALE
                * np.concatenate(
                    [W_in[:, :D_INNER][:, perm],
                     W_in[:, D_INNER + half * DH : D_INNER + (half + 1) * DH]],
                    axis=1,
                )
            ).astype(FP8NP),
            "convw": np.ascontiguousarray(
                cw.reshape(MT_XC, P, D_CONV).transpose(1, 0, 2).reshape(P, MT_XC * D_CONV)
            ),
            "convb": np.ascontiguousarray(conv_b[perm].reshape(MT_XC, P).T),
            "wx": np.ascontiguousarray(W_x[perm]).astype(BF16NP),
            "wdt": np.ascontiguousarray(W_dt[:, half * DH : (half + 1) * DH]).astype(BF16NP),
            "bdt": col_tiles(b_dt[half * DH : (half + 1) * DH]),
            "dvec": col_tiles(Dp[half * DH : (half + 1) * DH]),
            "wout": np.ascontiguousarray(W_out[half * DH : (half + 1) * DH]).astype(BF16NP),
            "gamma": np.ascontiguousarray(gamma[None, :]).astype(BF16NP),
            "beta": np.ascontiguousarray(beta[None, :]).astype(BF16NP),
            "eye": np.eye(P, dtype=np.float32).astype(BF16NP),
        }
        in_maps.append(m)
    return in_maps


def derive_a_n(inputs):
    A_log = np.asarray(inputs["A_log"], np.float32)
    return tuple(float(v) for v in (-np.exp(A_log[0, :])))


_PROGRAM_CACHE = {}


def get_program(a_n):
    key = a_n
    if key not in _PROGRAM_CACHE:
        _PROGRAM_CACHE[key] = build_program(a_n)
    return _PROGRAM_CACHE[key]


def assemble(results):
    out = np.empty((BATCH, SEQ, D_MODEL), np.float32)
    for c in range(N_CORES):
        b, half = divmod(c, 2)
        r = results[c]["out_half"]
        out[b, half * LQ : (half + 1) * LQ] = r[0:LQ]
        out[b, 512 + half * LQ : 512 + (half + 1) * LQ] = r[LQ : 2 * LQ]
    return out


def kernel(**inputs):
    from concourse import bass_utils

    a_n = derive_a_n(inputs)
    nc = get_program(a_n)
    in_maps = _shard_inputs(inputs)
    res = bass_utils.run_bass_kernel_spmd(nc, in_maps, core_ids=list(range(N_CORES)))
    return assemble(res.results)



# revision 2
# speedup vs baseline: 1.0131x; 1.0131x over previous
"""MoE-Mamba block kernel for 8 Trainium2 NeuronCores — perf rework v3.

Sharding: core c = (batch b = c//2, d_inner half = c%2). 763us -> ~690us.

Key structure (trace-driven; DVE vector engine is the bottleneck):
- Mega-batched selective scan: ONE tensor_tensor_scan per (m-tile,
  L-chunk, 8-state group) with zero-da seed columns between states
  (h = 0*h + dbu_seed resets the recurrence between states), cutting
  256 scans + 512 elementwise TTs to 48 scans + 96 wide 2x-mode TTs.
  Chunk carries inject/extract through seed columns with tiny
  same-engine copies.
- dbu built as one wide TT per group: dtu broadcast across states via
  an outer 0-stride AP (inner dim stays packed, so the 2x perf mode
  holds); B/C state-stacked broadcast tiles.
- Three L-chunks (512/256/256): each chunk's out_proj/ReduceScatter/
  LayerNorm is emitted inside the next chunk's scan stream and hides
  under it; only the last 256-row chunk's tail is exposed.
- Stage A computes only the core's own 1024 xc channels; the x_proj
  partial [96,1024] is pair-AllReduced (196KB bf16) instead of
  duplicating the other half's in_proj+conv. A tiny dummy AllReduce
  at kernel start absorbs the first-collective setup latency.
- B/C broadcasts: GP partition_broadcast for chunk 0 (head, DVE idle);
  PE ones-matmul + ACT psum copy for chunks 1/2, sliced into the
  previous chunk's per-cycle ACT slack (GP broadcasts derate
  concurrent DVE ops ~3x, so GP stays idle during the scan).
- dt_proj softplus = Ln(exp(x)+1) batched all-Exps-then-all-Lns on the
  shared natural_log_exp table (no table thrash in the scan phase).
- fp8 in_proj/conv (host-prescaled weights, diag conv tiles), fp8 z,
  D-term as a diag(D) PE matmul into the y PSUM accumulator, LeakyReLU
  as a DVE scalar_tensor_tensor, bf16 residual/LN datapath.
"""

import os
import sys

import numpy as np

try:
    import ml_dtypes
except ImportError:  # pragma: no cover
    ml_dtypes = None


def _ensure_import():
    try:
        import concourse  # noqa: F401
    except ImportError:
        for p in ("/opt/trn_rl_repo", os.path.expanduser("~/.axon_site/_ro/trn_rl_repo")):
            if os.path.isdir(p):
                sys.path.insert(0, p)
                break


_ensure_import()
os.environ.setdefault("MYCRO_LOCAL_CACHE", "1")

from contextlib import ExitStack  # noqa: E402

import concourse.bass as bass  # noqa: E402
import concourse.tile as tile  # noqa: E402
from concourse import bacc, mybir  # noqa: E402

F32 = mybir.dt.float32
BF16 = mybir.dt.bfloat16
FP8 = mybir.dt.float8e4
AF = mybir.ActivationFunctionType
OP = mybir.AluOpType

D_MODEL = 1024
D_INNER = 2048
D_STATE = 16
D_CONV = 4
DT_RANK = 64
BATCH = 4
SEQ = 1024
DH = D_INNER // 2  # channels per core
P = 128
KT = D_MODEL // P          # 8  k-tiles over d_model
MH = DH // P               # 8  own xc / z / scan d-tiles per core
LQ = SEQ // 4              # 256 rows per collective-half per core
LN_EPS = 1e-5
LRELU = 0.01
WSCALE = 16.0              # exact power-of-two rescale for fp8 in_proj weights
CSCALE = 64.0              # power-of-two rescale for fp8 conv weights
NCH = 3                    # L-chunks
CW = [512, 256, 256]       # l per chunk
CS = [0, 512, 768]         # chunk start l
SC = [c + 1 for c in CW]   # seed col + values per (state, chunk)
SCOFF = [0, 513, 770]      # chunk col offset in seeded full-L tiles
SEQS = SEQ + NCH           # 1027 cols in seeded full-L tiles
NG = 2                     # state groups
GS = D_STATE // NG         # 8 states per group
GW0 = GS * SC[0]           # group tile cols sized for the largest chunk
BIGPAD = 1.0e30            # dt pad value; exp(a_n * BIGPAD) == 0 resets the scan

N_CORES = 8
REPLICA_GROUPS = [[0, 1], [2, 3], [4, 5], [6, 7]]


def build_program(a_n, enable_asserts=False):
    """Build + compile the single-core SPMD Bass program. a_n: 16 floats."""
    nc = bacc.Bacc(
        "TRN2",
        target_bir_lowering=False,
        debug=False,
        enable_asserts=enable_asserts,
        num_devices=N_CORES,
    )

    # ---- I/O declarations (per-core shards; names match _shard_inputs) ----
    xt_d = nc.dram_tensor("xt", [D_MODEL, SEQ], FP8, kind="ExternalInput").ap()
    xres_d = nc.dram_tensor("xres", [2 * LQ, D_MODEL], BF16, kind="ExternalInput").ap()
    win_d = nc.dram_tensor("win", [D_MODEL, 2 * DH], FP8, kind="ExternalInput").ap()
    cd_d = nc.dram_tensor("convdiag", [P, MH * D_CONV * P], FP8, kind="ExternalInput").ap()
    cb_d = nc.dram_tensor("convb", [P, MH], F32, kind="ExternalInput").ap()
    wx_d = nc.dram_tensor("wx", [DH, DT_RANK + 2 * D_STATE], BF16, kind="ExternalInput").ap()
    wdt_d = nc.dram_tensor("wdt", [DT_RANK, DH], BF16, kind="ExternalInput").ap()
    bdt_d = nc.dram_tensor("bdt", [P, MH], F32, kind="ExternalInput").ap()
    dvdg_d = nc.dram_tensor("dvdiag", [P, MH * P], BF16, kind="ExternalInput").ap()
    wout_d = nc.dram_tensor("wout", [DH, D_MODEL], BF16, kind="ExternalInput").ap()
    gamma_d = nc.dram_tensor("gamma", [1, D_MODEL], BF16, kind="ExternalInput").ap()
    beta_d = nc.dram_tensor("beta", [1, D_MODEL], BF16, kind="ExternalInput").ap()
    eye_d = nc.dram_tensor("eye", [P, P], BF16, kind="ExternalInput").ap()
    out_d = nc.dram_tensor("out_half", [2 * LQ, D_MODEL], F32, kind="ExternalOutput").ap()

    NPROJ = DT_RANK + 2 * D_STATE  # 96

    with tile.TileContext(nc) as tc, ExitStack() as es:
        pers = es.enter_context(tc.tile_pool(name="pers", bufs=1))
        ps = es.enter_context(tc.tile_pool(name="psum", bufs=3, space="PSUM"))
        dram = es.enter_context(tc.tile_pool(name="dram", bufs=1, space="DRAM"))

        # ---- small constants ----
        cb_sb = pers.tile([P, MH], F32, name="cb_sb")
        nc.sync.dma_start(cb_sb[:], cb_d[:])
        bdt_sb = pers.tile([P, MH], F32, name="bdt_sb")
        nc.sync.dma_start(bdt_sb[:], bdt_d[:])
        dvdg = pers.tile([P, MH * P], BF16, name="dvdg")
        nc.sync.dma_start(dvdg[:], dvdg_d[:])
        eye16 = pers.tile([P, P], BF16, name="eye16")
        nc.sync.dma_start(eye16[:], eye_d[:])
        g16 = pers.tile([1, D_MODEL], BF16, name="g16")
        nc.sync.dma_start(g16[:], gamma_d[:])
        b16 = pers.tile([1, D_MODEL], BF16, name="b16")
        nc.sync.dma_start(b16[:], beta_d[:])
        gb_g = pers.tile([P, D_MODEL], BF16, name="gb_g")
        nc.gpsimd.partition_broadcast(gb_g[:], g16[0:1, :])
        gb_b = pers.tile([P, D_MODEL], BF16, name="gb_b")
        nc.gpsimd.partition_broadcast(gb_b[:], b16[0:1, :])

        # ---- medium-lived tensors ----
        # pair-AllReduced x_proj result; rows 0:64 = dt_raw, 64:96 = B/C
        xps = pers.tile([NPROJ, SEQ], BF16, name="xps")
        ones1 = pers.tile([1, P], BF16, name="ones1")
        nc.vector.memset(ones1[:], 1.0)
        wx16 = [pers.tile([P, NPROJ], BF16, name=f"wx16_{k}") for k in range(MH)]
        wdt16 = pers.tile([DT_RANK, DH], BF16, name="wdt16")

        p_ug = es.enter_context(tc.tile_pool(name="p_ug", bufs=1))  # until gating
        u16m = [p_ug.tile([P, SEQ], BF16, name=f"u16m_{m}") for m in range(MH)]
        zs8 = [p_ug.tile([P, SEQ], FP8, name=f"zs8_{m}") for m in range(MH)]

        p_y = es.enter_context(tc.tile_pool(name="p_y", bufs=1))
        p_bc = es.enter_context(tc.tile_pool(name="p_bc", bufs=1))

        xp_in = dram.tile([NPROJ, SEQ], BF16, name="xp_in")
        xp_out = dram.tile([NPROJ, SEQ], BF16, name="xp_out")
        warm_d = dram.tile([1, 8], BF16, name="cc_warm")
        nc.gpsimd.collective_compute(
            "AllReduce", OP.add, replica_groups=REPLICA_GROUPS,
            ins=[warm_d.opt()], outs=[warm_d.opt()],
        )

        # =========== stage A: in_proj (fp8, own half) + conv + x_proj ===========
        with (
            tc.tile_pool(name="p_xw", bufs=1) as pxw,
            tc.tile_pool(name="p_conv", bufs=1) as pconv,
        ):
            xt8 = []
            w8 = []
            for k in range(KT):
                t = pxw.tile([P, SEQ], FP8, name=f"xt8_{k}")
                nc.sync.dma_start(t[:], xt_d[k * P : (k + 1) * P, :])
                xt8.append(t)
                w = pxw.tile([P, 2 * DH], FP8, name=f"w8_{k}")
                nc.sync.dma_start(w[:], win_d[k * P : (k + 1) * P, :])
                w8.append(w)
            cdiag = pxw.tile([P, MH * D_CONV * P], FP8, name="cdiag")
            nc.sync.dma_start(cdiag[:], cd_d[:])
            for k in range(MH):
                nc.sync.dma_start(wx16[k][:], wx_d[k * P : (k + 1) * P, :])
            nc.sync.dma_start(wdt16[:], wdt_d[:])

            px = [ps.tile([NPROJ, SEQ // 2], F32, tag="xp", bufs=2, name=f"px_{i}")
                  for i in range(2)]

            def in_proj_tile(mt):
                """One [128, SEQ] column tile of x @ W_in -> PSUM pair (fp8)."""
                pts = [ps.tile([P, SEQ // 2], F32, tag="mm", bufs=4, name=f"pt_{mt}_{i}")
                       for i in range(2)]
                for k in range(KT):
                    for lhv in range(2):
                        nc.tensor.matmul(
                            pts[lhv][:],
                            w8[k][:, mt * P : (mt + 1) * P],
                            xt8[k][:, lhv * 512 : (lhv + 1) * 512],
                            start=(k == 0),
                            stop=(k == KT - 1),
                        )
                return pts

            # --- own xc half: in_proj -> conv -> silu -> x_proj partial ---
            xcp = [pconv.tile([P, SEQ + 4], FP8, name=f"xcp_{m}") for m in range(MH)]
            for mt in range(MH):
                dst = xcp[mt]
                nc.vector.memset(dst[:, 0:4], 0.0)
                pts = in_proj_tile(mt)
                for lhv in range(2):
                    nc.scalar.activation(
                        dst[:, 4 + lhv * 512 : 4 + (lhv + 1) * 512], pts[lhv][:],
                        AF.Copy, scale=1.0 / WSCALE,
                    )
                pus = [ps.tile([P, SEQ // 2], F32, tag="mm", bufs=4, name=f"pu_{mt}_{i}")
                       for i in range(2)]
                for j in range(D_CONV):
                    dg = cdiag[:, (mt * D_CONV + j) * P : (mt * D_CONV + j + 1) * P]
                    for lhv in range(2):
                        nc.tensor.matmul(
                            pus[lhv][:],
                            dg,
                            xcp[mt][:, 1 + j + lhv * 512 : 1 + j + lhv * 512 + 512],
                            start=(j == 0),
                            stop=(j == D_CONV - 1),
                        )
                ut = u16m[mt]
                for lhv in range(2):
                    nc.scalar.activation(
                        ut[:, lhv * 512 : (lhv + 1) * 512], pus[lhv][:],
                        AF.Silu, bias=cb_sb[:, mt : mt + 1], scale=1.0 / CSCALE,
                    )
                for lhv in range(2):
                    nc.tensor.matmul(
                        px[lhv][:],
                        wx16[mt][:],
                        ut[:, lhv * 512 : (lhv + 1) * 512],
                        start=(mt == 0),
                        stop=(mt == MH - 1),
                    )

            # --- drain partial x_proj -> DRAM -> pair AllReduce -> xps ---
            pxsb = pconv.tile([NPROJ, SEQ], BF16, name="pxsb")
            for lhv in range(2):
                nc.scalar.activation(
                    pxsb[:, lhv * 512 : (lhv + 1) * 512], px[lhv][:], AF.Copy
                )
            nc.sync.dma_start(xp_in[:], pxsb[:])
            nc.gpsimd.collective_compute(
                "AllReduce",
                OP.add,
                replica_groups=REPLICA_GROUPS,
                ins=[xp_in.opt()],
                outs=[xp_out.opt()],
            )
            nc.sync.dma_start(xps[:], xp_out[:])

            # --- B/C broadcasts on GP into state-stacked group tiles ---
            bbg = [[None] * NG for _ in range(NCH)]
            ccg = [[None] * NG for _ in range(NCH)]
            for c in range(NCH):
                for g in range(NG):
                    bbg[c][g] = p_bc.tile([P, GW0], BF16, name=f"bb_{c}_{g}",
                                          tag=f"bb_{g}", bufs=2)
                    ccg[c][g] = p_bc.tile([P, GW0], BF16, name=f"cc_{c}_{g}",
                                          tag=f"cc_{g}", bufs=1)

            def bcast_group(c, g, kind, use_gp):
                sc, cw = SC[c], CW[c]
                dstt = (bbg if kind == "b" else ccg)[c][g]
                nc.vector.memset(
                    dstt[:, 0 : GS * sc].rearrange(
                        "p (s n) -> p s n", n=sc)[:, :, 0], 0.0,
                )
                for i in range(GS):
                    n = g * GS + i + (DT_RANK if kind == "b"
                                      else DT_RANK + D_STATE)
                    if use_gp:
                        row = p_bc.tile([1, cw], BF16, tag=f"row{cw}", bufs=2)
                        nc.sync.dma_start(
                            row[:], xps[n : n + 1, CS[c] : CS[c] + cw]
                        )
                        nc.gpsimd.partition_broadcast(
                            dstt[:, i * sc + 1 : i * sc + 1 + cw], row[0:1, :]
                        )
                    else:
                        row = p_bc.tile([1, cw], BF16, tag=f"row{cw}", bufs=2)
                        nc.sync.dma_start(
                            row[:], xps[n : n + 1, CS[c] : CS[c] + cw]
                        )
                        bp = ps.tile([P, cw], F32, tag="mm", bufs=4,
                                     name=f"bp_{c}_{g}_{kind}_{i}")
                        nc.tensor.matmul(
                            bp[:], ones1[:], row[0:1, :],
                            start=True, stop=True,
                        )
                        nc.scalar.activation(
                            dstt[:, i * sc + 1 : i * sc + 1 + cw], bp[:], AF.Copy
                        )

            def bcast_chunk(c, use_gp=False):
                for g in range(NG):
                    for kind in ("b", "c"):
                        bcast_group(c, g, kind, use_gp)

            bcast_chunk(0, use_gp=True)

            # --- z half: in_proj -> silu (overlaps AR + broadcasts) ---
            for mz in range(MH):
                pts = in_proj_tile(MH + mz)
                for lhv in range(2):
                    nc.scalar.activation(
                        zs8[mz][:, lhv * 512 : (lhv + 1) * 512], pts[lhv][:],
                        AF.Silu, scale=1.0 / WSCALE,
                    )

        # =========== stage C: mega-batched selective scan, 2 L-chunks ===========
        binb0 = dram.tile([CW[0], D_MODEL], BF16, name="bounce_in0")
        bout0 = dram.tile([LQ, D_MODEL], BF16, name="bounce_out0")
        binbq = [dram.tile([2 * P, D_MODEL], BF16, name=f"bounce_in{c}") for c in (1, 2)]
        boutq = [dram.tile([P, D_MODEL], BF16, name=f"bounce_out{c}") for c in (1, 2)]

        p_dt = es.enter_context(tc.tile_pool(name="p_dt", bufs=1))
        dt16 = [p_dt.tile([P, SEQS], BF16, name=f"dt16_{m}") for m in range(MH)]
        dtu16 = [p_dt.tile([P, SEQS], BF16, name=f"dtu16_{m}") for m in range(MH)]
        carry = [p_dt.tile([P, D_STATE], BF16, name=f"carry_{m}") for m in range(MH)]

        psc = es.enter_context(tc.tile_pool(name="p_scan", bufs=1))
        pw = es.enter_context(tc.tile_pool(name="p_wo", bufs=1))
        pop = es.enter_context(tc.tile_pool(name="p_op", bufs=1))
        pln = es.enter_context(tc.tile_pool(name="p_ln", bufs=1))

        y16 = [[None] * MH for _ in range(NCH)]

        # --- dt_proj for all m upfront, softplus = Ln(exp(raw + b_dt) + 1).
        # Exp and Ln share the natural_log_exp table; batching all Exps then
        # all Lns keeps the scan phase on one table (no thrash). The exp
        # temporary g is staged in dtu16[m], overwritten by dt*u afterwards.
        # seeded segment list: (tile col start, seq col start, width)
        SEGS = [(SCOFF[c] + 1, CS[c], CW[c]) for c in range(NCH)]
        for m in range(MH):
            dt = dt16[m]
            for c in range(NCH):
                nc.vector.memset(dt[:, SCOFF[c] : SCOFF[c] + 1], BIGPAD)
            for lhv in range(2):
                pt = ps.tile([P, SEQ // 2], F32, tag="mm", bufs=4)
                nc.tensor.matmul(
                    pt[:],
                    wdt16[:, m * P : (m + 1) * P],
                    xps[0:DT_RANK, lhv * 512 : (lhv + 1) * 512],
                    start=True,
                    stop=True,
                )
            # lhv 0 = chunk 0; lhv 1 = chunks 1+2 (split the 512-col psum)
                if lhv == 0:
                    nc.scalar.activation(
                        dtu16[m][:, 1 : 1 + CW[0]], pt[:], AF.Exp,
                        bias=bdt_sb[:, m : m + 1], scale=1.0,
                    )
                else:
                    for c in (1, 2):
                        nc.scalar.activation(
                            dtu16[m][:, SCOFF[c] + 1 : SCOFF[c] + 1 + CW[c]],
                            pt[:, CS[c] - 512 : CS[c] - 512 + CW[c]], AF.Exp,
                            bias=bdt_sb[:, m : m + 1], scale=1.0,
                        )
        def make_dtu(m):
            dtu = dtu16[m]
            for c in range(NCH):
                nc.vector.memset(dtu[:, SCOFF[c] : SCOFF[c] + 1], 0.0)
            for tc0, sc0, w in SEGS:
                nc.vector.tensor_tensor(
                    dtu[:, tc0 : tc0 + w],
                    dt16[m][:, tc0 : tc0 + w],
                    u16m[m][:, sc0 : sc0 + w],
                    op=OP.mult,
                )

        for m in range(MH):
            dt = dt16[m]
            for tc0, sc0, w in SEGS:
                nc.scalar.activation(
                    dt[:, tc0 : tc0 + w],
                    dtu16[m][:, tc0 : tc0 + w], AF.Ln, bias=1.0,
                )
        make_dtu(0)
        make_dtu(1)

        def scan_m(c, m, defer_gate=False):
            """scan + gating for one (chunk, m-tile)."""
            sc, cw, off = SC[c], CW[c], SCOFF[c]
            gw = GS * sc
            yps = ps.tile([P, 512], F32, tag="ymm", bufs=2, name=f"yp_{c}_{m}")
            ypsum = yps[:, 0:cw]
            for g in range(NG):
                da = psc.tile([P, GW0], BF16, tag="da", bufs=2)
                for i in range(GS):
                    n = g * GS + i
                    nc.scalar.activation(
                        da[:, i * sc : (i + 1) * sc],
                        dt16[m][:, off : off + sc],
                        AF.Exp, scale=float(a_n[n]),
                    )
                dbu = psc.tile([P, GW0], BF16, tag="dh", bufs=2)
                dtu_b = (
                    dtu16[m][:, off : off + sc]
                    .unsqueeze(1)
                    .broadcast_to([P, GS, sc])
                )
                nc.vector.tensor_tensor(
                    dbu[:, 0:gw].rearrange("p (s n) -> p s n", n=sc),
                    dtu_b,
                    bbg[c][g][:, 0:gw].rearrange("p (s n) -> p s n", n=sc),
                    op=OP.mult,
                )
                if c > 0:
                    nc.vector.tensor_scalar(
                        dbu[:, 0:gw].rearrange("p (s n) -> p s n", n=sc)[:, :, 0],
                        carry[m][:, g * GS : (g + 1) * GS], 1.0, None, op0=OP.mult,
                    )
                h = psc.tile([P, GW0], BF16, tag="dh", bufs=2)
                nc.vector.tensor_tensor_scan(
                    h[:, 0:gw], da[:, 0:gw], dbu[:, 0:gw], 0.0, op0=OP.mult, op1=OP.add
                )
                if c < NCH - 1:
                    nc.vector.tensor_scalar(
                        carry[m][:, g * GS : (g + 1) * GS],
                        h[:, 0:gw].rearrange("p (s n) -> p s n", n=sc)[:, :, cw],
                        1.0, None, op0=OP.mult,
                    )
                yt = psc.tile([P, GW0], BF16, tag="yt", bufs=1)
                nc.vector.tensor_tensor(
                    yt[:, 0:gw], h[:, 0:gw], ccg[c][g][:, 0:gw], op=OP.mult
                )
                for i in range(GS):
                    nc.tensor.matmul(
                        ypsum,
                        eye16[:],
                        yt[:, i * sc + 1 : i * sc + 1 + cw],
                        start=(g == 0 and i == 0),
                        stop=False,
                    )
            nc.tensor.matmul(
                ypsum,
                dvdg[:, m * P : (m + 1) * P],
                u16m[m][:, CS[c] : CS[c] + cw],
                start=False,
                stop=True,
            )
            def gate():
                yt16 = p_y.tile([P, cw], BF16, name=f"y16_{c}_{m}",
                                tag=(f"y0_{m}" if c == 0 else f"yq_{m}"),
                                bufs=1)
                nc.vector.tensor_tensor(
                    yt16[:], ypsum, zs8[m][:, CS[c] : CS[c] + cw], op=OP.mult
                )
                y16[c][m] = yt16
            if defer_gate:
                return gate
            gate()
            return None

        def load_wout():
            ts = []
            for k in range(MH):
                t = pw.tile([P, D_MODEL], BF16, tag=f"wo_{k}", bufs=1)
                nc.sync.dma_start(t[:], wout_d[k * P : (k + 1) * P, :])
                ts.append(t)
            return ts

        def out_proj_lt(c, lt, dst, dst_row, wout16):
            """One 128-row block of y[c] @ W_out -> dst DRAM rows."""
            op_sb = pop.tile([P, D_MODEL], BF16, tag="op")
            pts = [ps.tile([P, 512], F32, tag="mm", bufs=4, name=f"po_{c}_{lt}_{i}")
                   for i in range(2)]
            for k in range(MH):
                for nf in range(2):
                    nc.tensor.matmul(
                        pts[nf][:],
                        y16[c][k][:, lt * P : (lt + 1) * P],
                        wout16[k][:, nf * 512 : (nf + 1) * 512],
                        start=(k == 0),
                        stop=(k == MH - 1),
                    )
            for nf in range(2):
                nc.scalar.activation(
                    op_sb[:, nf * 512 : (nf + 1) * 512], pts[nf][:], AF.Copy
                )
            nc.sync.dma_start(dst[dst_row * P : (dst_row + 1) * P, :], op_sb[:])

        def ln_tile(src, src_row, out_row):
            """Residual + LayerNorm + LeakyReLU for one 128-row tile."""
            h16 = pln.tile([P, D_MODEL], BF16, tag="h16")
            nc.sync.dma_start(h16[:], src[src_row * P : (src_row + 1) * P, :])
            xr = pln.tile([P, D_MODEL], BF16, tag="xr")
            nc.sync.dma_start(xr[:], xres_d[out_row * P : (out_row + 1) * P, :])
            hres = pln.tile([P, D_MODEL], BF16, tag="hres")
            nc.vector.tensor_tensor(hres[:], h16[:], xr[:], op=OP.add)
            ssum = pln.tile([P, 1], F32, tag="ssum")
            nc.vector.tensor_reduce(ssum[:], hres[:], axis=mybir.AxisListType.X, op=OP.add)
            sq = pln.tile([P, D_MODEL], F32, tag="sq")
            ssq = pln.tile([P, 1], F32, tag="ssq")
            nc.scalar.activation(sq[:], hres[:], AF.Square, accum_out=ssq[:])
            mu = pln.tile([P, 1], F32, tag="mu")
            nc.vector.tensor_scalar(mu[:], ssum[:], 1.0 / D_MODEL, None, op0=OP.mult)
            v = pln.tile([P, 1], F32, tag="v")
            nc.vector.tensor_tensor(v[:], mu[:], mu[:], op=OP.mult)
            nc.vector.scalar_tensor_tensor(
                v[:], ssq[:], 1.0 / D_MODEL, v[:], op0=OP.mult, op1=OP.subtract
            )
            nc.vector.tensor_scalar(v[:], v[:], LN_EPS, None, op0=OP.add)
            rv = pln.tile([P, 1], F32, tag="rv")
            nc.vector.reciprocal(rv[:], v[:])
            rs = pln.tile([P, 1], F32, tag="rs")
            nc.scalar.activation(rs[:], rv[:], AF.Sqrt)
            nb = pln.tile([P, 1], F32, tag="nb")
            nc.vector.tensor_tensor(nb[:], mu[:], rs[:], op=OP.mult)
            nc.vector.tensor_scalar(nb[:], nb[:], -1.0, None, op0=OP.mult)
            xn = pln.tile([P, D_MODEL], BF16, tag="h16")
            nc.scalar.activation(
                xn[:], hres[:], AF.Identity, bias=nb[:], scale=rs[:]
            )
            nc.vector.tensor_tensor(xn[:], xn[:], gb_g[:], op=OP.mult)
            fin = pln.tile([P, D_MODEL], BF16, tag="xr")
            nc.vector.tensor_tensor(fin[:], xn[:], gb_b[:], op=OP.add)
            fin2 = pln.tile([P, D_MODEL], F32, tag="sq")
            nc.vector.scalar_tensor_tensor(
                fin2[:], fin[:], LRELU, fin[:], op0=OP.mult, op1=OP.max
            )
            nc.sync.dma_start(out_d[out_row * P : (out_row + 1) * P, :], fin2[:])

        # chunk 0 scans; z tiles stream in behind the first scans, and
        # gating defers one m so it never stalls the DVE queue on zs8
        BC_SLOTS = [(0, "b"), (0, "c"), (1, "b"), (1, "c")]
        pending_gate = None
        for m in range(MH):
            g = scan_m(0, m, defer_gate=True)
            if m >= 2:
                make_dtu(m)
            if m % 2 == 1:
                grp, kind = BC_SLOTS[m // 2]
                bcast_group(1, grp, kind, False)
            if pending_gate is not None:
                pending_gate()
            pending_gate = g
        pending_gate()
        # keep the ACT/DVE queues fed before chunk-0 out_proj drains
        scan_m(1, 0)
        wout16 = load_wout()
        for lt in range(4):
            out_proj_lt(0, lt, binb0, lt, wout16)
        nc.gpsimd.collective_compute(
            "ReduceScatter", OP.add, replica_groups=REPLICA_GROUPS,
            ins=[binb0.opt()], outs=[bout0.opt()],
        )
        for m in range(1, 4):
            scan_m(1, m)
            if m % 2 == 1:
                grp, kind = BC_SLOTS[m // 2]
                bcast_group(2, grp, kind, False)
        ln_tile(bout0, 0, 0)
        for m in range(4, MH):
            scan_m(1, m)
            if m % 2 == 1:
                grp, kind = BC_SLOTS[m // 2]
                bcast_group(2, grp, kind, False)
        ln_tile(bout0, 1, 1)
        scan_m(2, 0)
        for lt in range(2):
            out_proj_lt(1, lt, binbq[0], lt, wout16)
        nc.gpsimd.collective_compute(
            "ReduceScatter", OP.add, replica_groups=REPLICA_GROUPS,
            ins=[binbq[0].opt()], outs=[boutq[0].opt()],
        )
        for m in range(1, 4):
            scan_m(2, m)
        ln_tile(boutq[0], 0, 2)
        for m in range(4, MH):
            scan_m(2, m)
        for lt in range(2):
            out_proj_lt(2, lt, binbq[1], lt, wout16)
        nc.gpsimd.collective_compute(
            "ReduceScatter", OP.add, replica_groups=REPLICA_GROUPS,
            ins=[binbq[1].opt()], outs=[boutq[1].opt()],
        )
        ln_tile(boutq[1], 0, 3)

    nc.compile()
    return nc


def _shard_inputs(inputs):
    x = np.asarray(inputs["x"], np.float32)
    W_in = np.asarray(inputs["W_in"], np.float32)
    conv_w = np.asarray(inputs["conv_w"], np.float32)
    conv_b = np.asarray(inputs["conv_b"], np.float32)
    W_x = np.asarray(inputs["W_x"], np.float32)
    W_dt = np.asarray(inputs["W_dt"], np.float32)
    b_dt = np.asarray(inputs["b_dt"], np.float32)
    Dp = np.asarray(inputs["D"], np.float32)
    W_out = np.asarray(inputs["W_out"], np.float32)
    gamma = np.asarray(inputs["gamma"], np.float32)
    beta = np.asarray(inputs["beta"], np.float32)

    def col_tiles(v):  # [DH] -> [P, MH] (tile-major columns)
        return np.ascontiguousarray(v.reshape(-1, P).T)

    FP8NP = ml_dtypes.float8_e4m3fn
    BF16NP = ml_dtypes.bfloat16

    in_maps = []
    for c in range(N_CORES):
        b, half = divmod(c, 2)
        own = np.arange(half * DH, (half + 1) * DH)
        cw = conv_w[own]  # [1024, 4]
        cdiag = np.zeros((P, MH * D_CONV * P), np.float32)
        for mt in range(MH):
            for j in range(D_CONV):
                blk = cdiag[:, (mt * D_CONV + j) * P : (mt * D_CONV + j + 1) * P]
                np.fill_diagonal(blk, cw[mt * P : (mt + 1) * P, j] * CSCALE)
        dvd = np.zeros((P, MH * P), np.float32)
        dloc = Dp[half * DH : (half + 1) * DH]
        for m in range(MH):
            np.fill_diagonal(dvd[:, m * P : (m + 1) * P], dloc[m * P : (m + 1) * P])
        rows = [x[b, half * LQ : (half + 1) * LQ],
                x[b, 512 + half * P : 512 + half * P + P],
                x[b, 768 + half * P : 768 + half * P + P]]
        m = {
            "xt": np.ascontiguousarray(x[b].T).astype(FP8NP),
            "xres": np.ascontiguousarray(np.concatenate(rows, axis=0)).astype(BF16NP),
            "win": np.ascontiguousarray(
                WSCALE
                * np.concatenate(
                    [W_in[:, half * DH : (half + 1) * DH],
                     W_in[:, D_INNER + half * DH : D_INNER + (half + 1) * DH]],
                    axis=1,
                )
            ).astype(FP8NP),
            "convdiag": cdiag.astype(FP8NP),
            "convb": np.ascontiguousarray(conv_b[own].reshape(MH, P).T),
            "wx": np.ascontiguousarray(W_x[own]).astype(BF16NP),
            "wdt": np.ascontiguousarray(W_dt[:, half * DH : (half + 1) * DH]).astype(BF16NP),
            "bdt": col_tiles(b_dt[half * DH : (half + 1) * DH]),
            "dvdiag": dvd.astype(BF16NP),
            "wout": np.ascontiguousarray(W_out[half * DH : (half + 1) * DH]).astype(BF16NP),
            "gamma": np.ascontiguousarray(gamma[None, :]).astype(BF16NP),
            "beta": np.ascontiguousarray(beta[None, :]).astype(BF16NP),
            "eye": np.eye(P, dtype=np.float32).astype(BF16NP),
        }
        in_maps.append(m)
    return in_maps


def derive_a_n(inputs):
    A_log = np.asarray(inputs["A_log"], np.float32)
    return tuple(float(v) for v in (-np.exp(A_log[0, :])))


_PROGRAM_CACHE = {}


def get_program(a_n):
    key = a_n
    if key not in _PROGRAM_CACHE:
        _PROGRAM_CACHE[key] = build_program(a_n)
    return _PROGRAM_CACHE[key]


def assemble(results):
    out = np.empty((BATCH, SEQ, D_MODEL), np.float32)
    for c in range(N_CORES):
        b, half = divmod(c, 2)
        r = results[c]["out_half"]
        out[b, half * LQ : (half + 1) * LQ] = r[0:LQ]
        out[b, 512 + half * P : 512 + half * P + P] = r[LQ : LQ + P]
        out[b, 768 + half * P : 768 + half * P + P] = r[LQ + P : LQ + 2 * P]
    return out


def kernel(**inputs):
    from concourse import bass_utils

    a_n = derive_a_n(inputs)
    nc = get_program(a_n)
    in_maps = _shard_inputs(inputs)
    res = bass_utils.run_bass_kernel_spmd(nc, in_maps, core_ids=list(range(N_CORES)))
    return assemble(res.results)


# revision 5
# speedup vs baseline: 1.0314x; 1.0180x over previous
"""MoE-Mamba block kernel for 8 Trainium2 NeuronCores — perf rework v3.

Sharding: core c = (batch b = c//2, d_inner half = c%2). 763us -> ~690us.

Key structure (trace-driven; DVE vector engine is the bottleneck):
- Mega-batched selective scan: ONE tensor_tensor_scan per (m-tile,
  L-chunk, 8-state group) with zero-da seed columns between states
  (h = 0*h + dbu_seed resets the recurrence between states), cutting
  256 scans + 512 elementwise TTs to 48 scans + 96 wide 2x-mode TTs.
  Chunk carries inject/extract through seed columns with tiny
  same-engine copies.
- dbu built as one wide TT per group: dtu broadcast across states via
  an outer 0-stride AP (inner dim stays packed, so the 2x perf mode
  holds); B/C state-stacked broadcast tiles.
- Three L-chunks (512/256/256): each chunk's out_proj/ReduceScatter/
  LayerNorm is emitted inside the next chunk's scan stream and hides
  under it; only the last 256-row chunk's tail is exposed.
- Stage A computes only the core's own 1024 xc channels; the x_proj
  partial [96,1024] is pair-AllReduced (196KB bf16) instead of
  duplicating the other half's in_proj+conv. A tiny dummy AllReduce
  at kernel start absorbs the first-collective setup latency.
- B/C broadcasts: GP partition_broadcast for chunk 0 (head, DVE idle);
  PE ones-matmul + ACT psum copy for chunks 1/2, sliced into the
  previous chunk's per-cycle ACT slack (GP broadcasts derate
  concurrent DVE ops ~3x, so GP stays idle during the scan).
- dt_proj softplus = Ln(exp(x)+1) batched all-Exps-then-all-Lns on the
  shared natural_log_exp table (no table thrash in the scan phase).
- fp8 in_proj/conv (host-prescaled weights, diag conv tiles) with
  DoubleRow perf mode (two K-tiles contracted per fp8 matmul), fp8 z,
  D-term as a diag(D) PE matmul into the y PSUM accumulator, LeakyReLU
  as a DVE scalar_tensor_tensor, bf16 residual/LN datapath.
"""

import os
import sys

import numpy as np

try:
    import ml_dtypes
except ImportError:  # pragma: no cover
    ml_dtypes = None


def _ensure_import():
    try:
        import concourse  # noqa: F401
    except ImportError:
        for p in ("/opt/trn_rl_repo", os.path.expanduser("~/.axon_site/_ro/trn_rl_repo")):
            if os.path.isdir(p):
                sys.path.insert(0, p)
                break


_ensure_import()
os.environ.setdefault("MYCRO_LOCAL_CACHE", "1")

from contextlib import ExitStack  # noqa: E402

import concourse.bass as bass  # noqa: E402
import concourse.tile as tile  # noqa: E402
from concourse import bacc, mybir  # noqa: E402

F32 = mybir.dt.float32
BF16 = mybir.dt.bfloat16
FP8 = mybir.dt.float8e4
AF = mybir.ActivationFunctionType
OP = mybir.AluOpType

D_MODEL = 1024
D_INNER = 2048
D_STATE = 16
D_CONV = 4
DT_RANK = 64
BATCH = 4
SEQ = 1024
DH = D_INNER // 2  # channels per core
P = 128
KT = D_MODEL // P          # 8  k-tiles over d_model
MH = DH // P               # 8  own xc / z / scan d-tiles per core
LQ = SEQ // 4              # 256 rows per collective-half per core
LN_EPS = 1e-5
LRELU = 0.01
WSCALE = 16.0              # exact power-of-two rescale for fp8 in_proj weights
CSCALE = 64.0              # power-of-two rescale for fp8 conv weights
NCH = 3                    # L-chunks
CW = [512, 256, 256]       # l per chunk
CS = [0, 512, 768]         # chunk start l
SC = [c + 1 for c in CW]   # seed col + values per (state, chunk)
SCOFF = [0, 513, 770]      # chunk col offset in seeded full-L tiles
SEQS = SEQ + NCH           # 1027 cols in seeded full-L tiles
NG = 2                     # state groups
GS = D_STATE // NG         # 8 states per group
GW0 = GS * SC[0]           # group tile cols sized for the largest chunk
BIGPAD = 1.0e30            # dt pad value; exp(a_n * BIGPAD) == 0 resets the scan

N_CORES = 8
REPLICA_GROUPS = [[0, 1], [2, 3], [4, 5], [6, 7]]


def build_program(a_n, enable_asserts=False):
    """Build + compile the single-core SPMD Bass program. a_n: 16 floats."""
    nc = bacc.Bacc(
        "TRN2",
        target_bir_lowering=False,
        debug=False,
        enable_asserts=enable_asserts,
        num_devices=N_CORES,
    )

    # ---- I/O declarations (per-core shards; names match _shard_inputs) ----
    xt_d = nc.dram_tensor("xt", [D_MODEL, SEQ], FP8, kind="ExternalInput").ap()
    xres_d = nc.dram_tensor("xres", [2 * LQ, D_MODEL], BF16, kind="ExternalInput").ap()
    win_d = nc.dram_tensor("win", [D_MODEL, 2 * DH], FP8, kind="ExternalInput").ap()
    cd_d = nc.dram_tensor("convdiag", [P, MH * D_CONV * P], FP8, kind="ExternalInput").ap()
    cb_d = nc.dram_tensor("convb", [P, MH], F32, kind="ExternalInput").ap()
    wx_d = nc.dram_tensor("wx", [DH, DT_RANK + 2 * D_STATE], BF16, kind="ExternalInput").ap()
    wdt_d = nc.dram_tensor("wdt", [DT_RANK, DH], BF16, kind="ExternalInput").ap()
    bdt_d = nc.dram_tensor("bdt", [P, MH], F32, kind="ExternalInput").ap()
    dvdg_d = nc.dram_tensor("dvdiag", [P, MH * P], BF16, kind="ExternalInput").ap()
    wout_d = nc.dram_tensor("wout", [DH, D_MODEL], BF16, kind="ExternalInput").ap()
    gamma_d = nc.dram_tensor("gamma", [1, D_MODEL], BF16, kind="ExternalInput").ap()
    beta_d = nc.dram_tensor("beta", [1, D_MODEL], BF16, kind="ExternalInput").ap()
    eye_d = nc.dram_tensor("eye", [P, P], BF16, kind="ExternalInput").ap()
    out_d = nc.dram_tensor("out_half", [2 * LQ, D_MODEL], F32, kind="ExternalOutput").ap()

    NPROJ = DT_RANK + 2 * D_STATE  # 96

    with tile.TileContext(nc) as tc, ExitStack() as es:
        pers = es.enter_context(tc.tile_pool(name="pers", bufs=1))
        ps = es.enter_context(tc.tile_pool(name="psum", bufs=3, space="PSUM"))
        dram = es.enter_context(tc.tile_pool(name="dram", bufs=1, space="DRAM"))

        # ---- small constants ----
        cb_sb = pers.tile([P, MH], F32, name="cb_sb")
        nc.sync.dma_start(cb_sb[:], cb_d[:])
        bdt_sb = pers.tile([P, MH], F32, name="bdt_sb")
        nc.sync.dma_start(bdt_sb[:], bdt_d[:])
        dvdg = pers.tile([P, MH * P], BF16, name="dvdg")
        nc.sync.dma_start(dvdg[:], dvdg_d[:])
        eye16 = pers.tile([P, P], BF16, name="eye16")
        nc.sync.dma_start(eye16[:], eye_d[:])
        g16 = pers.tile([1, D_MODEL], BF16, name="g16")
        nc.sync.dma_start(g16[:], gamma_d[:])
        b16 = pers.tile([1, D_MODEL], BF16, name="b16")
        nc.sync.dma_start(b16[:], beta_d[:])
        gb_g = pers.tile([P, D_MODEL], BF16, name="gb_g")
        nc.gpsimd.partition_broadcast(gb_g[:], g16[0:1, :])
        gb_b = pers.tile([P, D_MODEL], BF16, name="gb_b")
        nc.gpsimd.partition_broadcast(gb_b[:], b16[0:1, :])

        # ---- medium-lived tensors ----
        # pair-AllReduced x_proj result; rows 0:64 = dt_raw, 64:96 = B/C
        xps = pers.tile([NPROJ, SEQ], BF16, name="xps")
        ones1 = pers.tile([1, P], BF16, name="ones1")
        nc.vector.memset(ones1[:], 1.0)
        wx16 = [pers.tile([P, NPROJ], BF16, name=f"wx16_{k}") for k in range(MH)]
        wdt16 = pers.tile([DT_RANK, DH], BF16, name="wdt16")

        p_ug = es.enter_context(tc.tile_pool(name="p_ug", bufs=1))  # until gating
        u16m = [p_ug.tile([P, SEQ], BF16, name=f"u16m_{m}") for m in range(MH)]
        zs8 = [p_ug.tile([P, SEQ], FP8, name=f"zs8_{m}") for m in range(MH)]

        p_y = es.enter_context(tc.tile_pool(name="p_y", bufs=1))
        p_bc = es.enter_context(tc.tile_pool(name="p_bc", bufs=1))

        xp_in = dram.tile([NPROJ, SEQ], BF16, name="xp_in")
        xp_out = dram.tile([NPROJ, SEQ], BF16, name="xp_out")
        warm_d = dram.tile([1, 8], BF16, name="cc_warm")
        nc.gpsimd.collective_compute(
            "AllReduce", OP.add, replica_groups=REPLICA_GROUPS,
            ins=[warm_d.opt()], outs=[warm_d.opt()],
        )

        # =========== stage A: in_proj (fp8, own half) + conv + x_proj ===========
        with (
            tc.tile_pool(name="p_xw", bufs=1) as pxw,
            tc.tile_pool(name="p_conv", bufs=1) as pconv,
        ):
            # k-tile PAIRS for fp8 DoubleRow matmuls (K=256 per instruction)
            xt8 = []
            w8 = []
            for k2 in range(KT // 2):
                t = pxw.tile([P, 2 * SEQ], FP8, name=f"xt8_{k2}")
                tv = t[:, :].rearrange("p (two f) -> p two f", two=2)
                w = pxw.tile([P, 2 * (2 * DH)], FP8, name=f"w8_{k2}")
                wv = w[:, :].rearrange("p (two f) -> p two f", two=2)
                for i in range(2):
                    r = (2 * k2 + i) * P
                    nc.sync.dma_start(tv[:, i, :], xt_d[r : r + P, :])
                    nc.sync.dma_start(wv[:, i, :], win_d[r : r + P, :])
                xt8.append(tv)
                w8.append(wv)
            cdiag = pxw.tile([P, MH * D_CONV * P], FP8, name="cdiag")
            nc.sync.dma_start(cdiag[:], cd_d[:])
            for k in range(MH):
                nc.sync.dma_start(wx16[k][:], wx_d[k * P : (k + 1) * P, :])
            nc.sync.dma_start(wdt16[:], wdt_d[:])

            px = [ps.tile([NPROJ, SEQ // 2], F32, tag="xp", bufs=2, name=f"px_{i}")
                  for i in range(2)]

            def in_proj_tile(mt):
                """One [128, SEQ] column tile of x @ W_in -> PSUM pair (fp8,
                DoubleRow: two k-tiles contracted per matmul)."""
                pts = [ps.tile([P, SEQ // 2], F32, tag="mm", bufs=4, name=f"pt_{mt}_{i}")
                       for i in range(2)]
                for k2 in range(KT // 2):
                    for lhv in range(2):
                        nc.tensor.matmul(
                            pts[lhv][:],
                            w8[k2][:, :, mt * P : (mt + 1) * P],
                            xt8[k2][:, :, lhv * 512 : (lhv + 1) * 512],
                            start=(k2 == 0),
                            stop=(k2 == KT // 2 - 1),
                            perf_mode=mybir.MatmulPerfMode.DoubleRow,
                        )
                return pts

            # --- own xc half: in_proj -> conv -> silu -> x_proj partial ---
            xcp = [pconv.tile([P, SEQ + 4], FP8, name=f"xcp_{m}") for m in range(MH)]
            for mt in range(MH):
                dst = xcp[mt]
                nc.vector.memset(dst[:, 0:4], 0.0)
                pts = in_proj_tile(mt)
                for lhv in range(2):
                    nc.scalar.activation(
                        dst[:, 4 + lhv * 512 : 4 + (lhv + 1) * 512], pts[lhv][:],
                        AF.Copy, scale=1.0 / WSCALE,
                    )
                pus = [ps.tile([P, SEQ // 2], F32, tag="mm", bufs=4, name=f"pu_{mt}_{i}")
                       for i in range(2)]
                for j in range(D_CONV):
                    dg = cdiag[:, (mt * D_CONV + j) * P : (mt * D_CONV + j + 1) * P]
                    for lhv in range(2):
                        nc.tensor.matmul(
                            pus[lhv][:],
                            dg,
                            xcp[mt][:, 1 + j + lhv * 512 : 1 + j + lhv * 512 + 512],
                            start=(j == 0),
                            stop=(j == D_CONV - 1),
                        )
                ut = u16m[mt]
                for lhv in range(2):
                    nc.scalar.activation(
                        ut[:, lhv * 512 : (lhv + 1) * 512], pus[lhv][:],
                        AF.Silu, bias=cb_sb[:, mt : mt + 1], scale=1.0 / CSCALE,
                    )
                for lhv in range(2):
                    nc.tensor.matmul(
                        px[lhv][:],
                        wx16[mt][:],
                        ut[:, lhv * 512 : (lhv + 1) * 512],
                        start=(mt == 0),
                        stop=(mt == MH - 1),
                    )

            # --- drain partial x_proj -> DRAM -> pair AllReduce -> xps ---
            pxsb = pconv.tile([NPROJ, SEQ], BF16, name="pxsb")
            for lhv in range(2):
                nc.scalar.activation(
                    pxsb[:, lhv * 512 : (lhv + 1) * 512], px[lhv][:], AF.Copy
                )
            nc.sync.dma_start(xp_in[:], pxsb[:])
            nc.gpsimd.collective_compute(
                "AllReduce",
                OP.add,
                replica_groups=REPLICA_GROUPS,
                ins=[xp_in.opt()],
                outs=[xp_out.opt()],
            )
            nc.sync.dma_start(xps[:], xp_out[:])

            # --- B/C broadcasts on GP into state-stacked group tiles ---
            bbg = [[None] * NG for _ in range(NCH)]
            ccg = [[None] * NG for _ in range(NCH)]
            for c in range(NCH):
                for g in range(NG):
                    bbg[c][g] = p_bc.tile([P, GW0], BF16, name=f"bb_{c}_{g}",
                                          tag=f"bb_{g}", bufs=2)
                    ccg[c][g] = p_bc.tile([P, GW0], BF16, name=f"cc_{c}_{g}",
                                          tag=f"cc_{g}", bufs=1)

            def bcast_group(c, g, kind, use_gp):
                sc, cw = SC[c], CW[c]
                dstt = (bbg if kind == "b" else ccg)[c][g]
                nc.vector.memset(
                    dstt[:, 0 : GS * sc].rearrange(
                        "p (s n) -> p s n", n=sc)[:, :, 0], 0.0,
                )
                for i in range(GS):
                    n = g * GS + i + (DT_RANK if kind == "b"
                                      else DT_RANK + D_STATE)
                    if use_gp:
                        row = p_bc.tile([1, cw], BF16, tag=f"row{cw}", bufs=2)
                        nc.sync.dma_start(
                            row[:], xps[n : n + 1, CS[c] : CS[c] + cw]
                        )
                        nc.gpsimd.partition_broadcast(
                            dstt[:, i * sc + 1 : i * sc + 1 + cw], row[0:1, :]
                        )
                    else:
                        row = p_bc.tile([1, cw], BF16, tag=f"row{cw}", bufs=2)
                        nc.sync.dma_start(
                            row[:], xps[n : n + 1, CS[c] : CS[c] + cw]
                        )
                        bp = ps.tile([P, cw], F32, tag="mm", bufs=4,
                                     name=f"bp_{c}_{g}_{kind}_{i}")
                        nc.tensor.matmul(
                            bp[:], ones1[:], row[0:1, :],
                            start=True, stop=True,
                        )
                        nc.scalar.activation(
                            dstt[:, i * sc + 1 : i * sc + 1 + cw], bp[:], AF.Copy
                        )

            def bcast_chunk(c, use_gp=False):
                # GP path (head): both B tiles first so the first cycle's
                # dbu(g1) doesn't wait behind the C broadcasts
                order = (
                    [(g, k) for k in ("b", "c") for g in range(NG)]
                    if use_gp
                    else [(g, k) for g in range(NG) for k in ("b", "c")]
                )
                for g, kind in order:
                    bcast_group(c, g, kind, use_gp)

            bcast_chunk(0, use_gp=True)

            # --- z half: in_proj -> silu (overlaps AR + broadcasts) ---
            for mz in range(MH):
                pts = in_proj_tile(MH + mz)
                for lhv in range(2):
                    nc.scalar.activation(
                        zs8[mz][:, lhv * 512 : (lhv + 1) * 512], pts[lhv][:],
                        AF.Silu, scale=1.0 / WSCALE,
                    )

        # =========== stage C: mega-batched selective scan, 2 L-chunks ===========
        binb0 = dram.tile([CW[0], D_MODEL], BF16, name="bounce_in0")
        bout0 = dram.tile([LQ, D_MODEL], BF16, name="bounce_out0")
        binbq = [dram.tile([2 * P, D_MODEL], BF16, name=f"bounce_in{c}") for c in (1, 2)]
        boutq = [dram.tile([P, D_MODEL], BF16, name=f"bounce_out{c}") for c in (1, 2)]

        p_dt = es.enter_context(tc.tile_pool(name="p_dt", bufs=1))
        dt16 = [p_dt.tile([P, SEQS], BF16, name=f"dt16_{m}") for m in range(MH)]
        dtu16 = [p_dt.tile([P, SEQS], BF16, name=f"dtu16_{m}") for m in range(MH)]
        carry = [p_dt.tile([P, D_STATE], BF16, name=f"carry_{m}") for m in range(MH)]

        psc = es.enter_context(tc.tile_pool(name="p_scan", bufs=1))
        pw = es.enter_context(tc.tile_pool(name="p_wo", bufs=1))
        pop = es.enter_context(tc.tile_pool(name="p_op", bufs=1))
        pln = es.enter_context(tc.tile_pool(name="p_ln", bufs=1))

        y16 = [[None] * MH for _ in range(NCH)]

        # --- dt_proj for all m upfront, softplus = Ln(exp(raw + b_dt) + 1).
        # Exp and Ln share the natural_log_exp table; batching all Exps then
        # all Lns keeps the scan phase on one table (no thrash). The exp
        # temporary g is staged in dtu16[m], overwritten by dt*u afterwards.
        # seeded segment list: (tile col start, seq col start, width)
        SEGS = [(SCOFF[c] + 1, CS[c], CW[c]) for c in range(NCH)]
        for m in range(MH):
            dt = dt16[m]
            for c in range(NCH):
                nc.vector.memset(dt[:, SCOFF[c] : SCOFF[c] + 1], BIGPAD)
            for lhv in range(2):
                pt = ps.tile([P, SEQ // 2], F32, tag="mm", bufs=4)
                nc.tensor.matmul(
                    pt[:],
                    wdt16[:, m * P : (m + 1) * P],
                    xps[0:DT_RANK, lhv * 512 : (lhv + 1) * 512],
                    start=True,
                    stop=True,
                )
            # lhv 0 = chunk 0; lhv 1 = chunks 1+2 (split the 512-col psum)
                if lhv == 0:
                    nc.scalar.activation(
                        dtu16[m][:, 1 : 1 + CW[0]], pt[:], AF.Exp,
                        bias=bdt_sb[:, m : m + 1], scale=1.0,
                    )
                else:
                    for c in (1, 2):
                        nc.scalar.activation(
                            dtu16[m][:, SCOFF[c] + 1 : SCOFF[c] + 1 + CW[c]],
                            pt[:, CS[c] - 512 : CS[c] - 512 + CW[c]], AF.Exp,
                            bias=bdt_sb[:, m : m + 1], scale=1.0,
                        )
        def make_dtu(m):
            dtu = dtu16[m]
            for c in range(NCH):
                nc.vector.memset(dtu[:, SCOFF[c] : SCOFF[c] + 1], 0.0)
            for tc0, sc0, w in SEGS:
                nc.vector.tensor_tensor(
                    dtu[:, tc0 : tc0 + w],
                    dt16[m][:, tc0 : tc0 + w],
                    u16m[m][:, sc0 : sc0 + w],
                    op=OP.mult,
                )

        def make_ln(m):
            dt = dt16[m]
            for tc0, sc0, w in SEGS:
                nc.scalar.activation(
                    dt[:, tc0 : tc0 + w],
                    dtu16[m][:, tc0 : tc0 + w], AF.Ln, bias=1.0,
                )

        for m in range(3):
            make_ln(m)
        make_dtu(0)
        make_dtu(1)

        def scan_m(c, m, defer_gate=False):
            """scan + gating for one (chunk, m-tile)."""
            sc, cw, off = SC[c], CW[c], SCOFF[c]
            gw = GS * sc
            yps = ps.tile([P, 512], F32, tag="ymm", bufs=2, name=f"yp_{c}_{m}")
            ypsum = yps[:, 0:cw]
            for g in range(NG):
                da = psc.tile([P, GW0], BF16, tag="da", bufs=2)
                for i in range(GS):
                    n = g * GS + i
                    nc.scalar.activation(
                        da[:, i * sc : (i + 1) * sc],
                        dt16[m][:, off : off + sc],
                        AF.Exp, scale=float(a_n[n]),
                    )
                dbu = psc.tile([P, GW0], BF16, tag="dh", bufs=2)
                dtu_b = (
                    dtu16[m][:, off : off + sc]
                    .unsqueeze(1)
                    .broadcast_to([P, GS, sc])
                )
                nc.vector.tensor_tensor(
                    dbu[:, 0:gw].rearrange("p (s n) -> p s n", n=sc),
                    dtu_b,
                    bbg[c][g][:, 0:gw].rearrange("p (s n) -> p s n", n=sc),
                    op=OP.mult,
                )
                if c > 0:
                    nc.vector.tensor_scalar(
                        dbu[:, 0:gw].rearrange("p (s n) -> p s n", n=sc)[:, :, 0],
                        carry[m][:, g * GS : (g + 1) * GS], 1.0, None, op0=OP.mult,
                    )
                h = psc.tile([P, GW0], BF16, tag="dh", bufs=2)
                nc.vector.tensor_tensor_scan(
                    h[:, 0:gw], da[:, 0:gw], dbu[:, 0:gw], 0.0, op0=OP.mult, op1=OP.add
                )
                if c < NCH - 1:
                    nc.vector.tensor_scalar(
                        carry[m][:, g * GS : (g + 1) * GS],
                        h[:, 0:gw].rearrange("p (s n) -> p s n", n=sc)[:, :, cw],
                        1.0, None, op0=OP.mult,
                    )
                yt = psc.tile([P, GW0], BF16, tag="yt", bufs=1)
                nc.vector.tensor_tensor(
                    yt[:, 0:gw], h[:, 0:gw], ccg[c][g][:, 0:gw], op=OP.mult
                )
                for i in range(GS):
                    nc.tensor.matmul(
                        ypsum,
                        eye16[:],
                        yt[:, i * sc + 1 : i * sc + 1 + cw],
                        start=(g == 0 and i == 0),
                        stop=False,
                    )
            nc.tensor.matmul(
                ypsum,
                dvdg[:, m * P : (m + 1) * P],
                u16m[m][:, CS[c] : CS[c] + cw],
                start=False,
                stop=True,
            )
            def gate():
                yt16 = p_y.tile([P, cw], BF16, name=f"y16_{c}_{m}",
                                tag=(f"y0_{m}" if c == 0 else f"yq_{m}"),
                                bufs=1)
                nc.vector.tensor_tensor(
                    yt16[:], ypsum, zs8[m][:, CS[c] : CS[c] + cw], op=OP.mult
                )
                y16[c][m] = yt16
            if defer_gate:
                return gate
            gate()
            return None

        def load_wout():
            ts = []
            for k in range(MH):
                t = pw.tile([P, D_MODEL], BF16, tag=f"wo_{k}", bufs=1)
                nc.sync.dma_start(t[:], wout_d[k * P : (k + 1) * P, :])
                ts.append(t)
            return ts

        def out_proj_lt(c, lt, dst, dst_row, wout16):
            """One 128-row block of y[c] @ W_out -> dst DRAM rows."""
            op_sb = pop.tile([P, D_MODEL], BF16, tag="op")
            pts = [ps.tile([P, 512], F32, tag="mm", bufs=4, name=f"po_{c}_{lt}_{i}")
                   for i in range(2)]
            for k in range(MH):
                for nf in range(2):
                    nc.tensor.matmul(
                        pts[nf][:],
                        y16[c][k][:, lt * P : (lt + 1) * P],
                        wout16[k][:, nf * 512 : (nf + 1) * 512],
                        start=(k == 0),
                        stop=(k == MH - 1),
                    )
            for nf in range(2):
                nc.scalar.activation(
                    op_sb[:, nf * 512 : (nf + 1) * 512], pts[nf][:], AF.Copy
                )
            nc.sync.dma_start(dst[dst_row * P : (dst_row + 1) * P, :], op_sb[:])

        def ln_tile(src, src_row, out_row):
            """Residual + LayerNorm + LeakyReLU for one 128-row tile."""
            h16 = pln.tile([P, D_MODEL], BF16, tag="h16")
            nc.sync.dma_start(h16[:], src[src_row * P : (src_row + 1) * P, :])
            xr = pln.tile([P, D_MODEL], BF16, tag="xr")
            nc.sync.dma_start(xr[:], xres_d[out_row * P : (out_row + 1) * P, :])
            hres = pln.tile([P, D_MODEL], BF16, tag="hres")
            nc.vector.tensor_tensor(hres[:], h16[:], xr[:], op=OP.add)
            ssum = pln.tile([P, 1], F32, tag="ssum")
            nc.vector.tensor_reduce(ssum[:], hres[:], axis=mybir.AxisListType.X, op=OP.add)
            sq = pln.tile([P, D_MODEL], F32, tag="sq")
            ssq = pln.tile([P, 1], F32, tag="ssq")
            nc.scalar.activation(sq[:], hres[:], AF.Square, accum_out=ssq[:])
            mu = pln.tile([P, 1], F32, tag="mu")
            nc.vector.tensor_scalar(mu[:], ssum[:], 1.0 / D_MODEL, None, op0=OP.mult)
            v = pln.tile([P, 1], F32, tag="v")
            nc.vector.tensor_tensor(v[:], mu[:], mu[:], op=OP.mult)
            nc.vector.scalar_tensor_tensor(
                v[:], ssq[:], 1.0 / D_MODEL, v[:], op0=OP.mult, op1=OP.subtract
            )
            nc.vector.tensor_scalar(v[:], v[:], LN_EPS, None, op0=OP.add)
            rv = pln.tile([P, 1], F32, tag="rv")
            nc.vector.reciprocal(rv[:], v[:])
            rs = pln.tile([P, 1], F32, tag="rs")
            nc.scalar.activation(rs[:], rv[:], AF.Sqrt)
            nb = pln.tile([P, 1], F32, tag="nb")
            nc.vector.tensor_tensor(nb[:], mu[:], rs[:], op=OP.mult)
            nc.vector.tensor_scalar(nb[:], nb[:], -1.0, None, op0=OP.mult)
            xn = pln.tile([P, D_MODEL], BF16, tag="h16")
            nc.scalar.activation(
                xn[:], hres[:], AF.Identity, bias=nb[:], scale=rs[:]
            )
            nc.vector.tensor_tensor(xn[:], xn[:], gb_g[:], op=OP.mult)
            fin = pln.tile([P, D_MODEL], BF16, tag="xr")
            nc.vector.tensor_tensor(fin[:], xn[:], gb_b[:], op=OP.add)
            fin2 = pln.tile([P, D_MODEL], F32, tag="sq")
            nc.vector.scalar_tensor_tensor(
                fin2[:], fin[:], LRELU, fin[:], op0=OP.mult, op1=OP.max
            )
            nc.sync.dma_start(out_d[out_row * P : (out_row + 1) * P, :], fin2[:])

        # chunk 0 scans; z tiles stream in behind the first scans, and
        # gating defers one m so it never stalls the DVE queue on zs8
        BC_SLOTS = [(0, "b"), (0, "c"), (1, "b"), (1, "c")]
        pending_gate = None
        for m in range(MH):
            g = scan_m(0, m, defer_gate=True)
            if m + 3 < MH:
                make_ln(m + 3)
            if m >= 2:
                make_dtu(m)
            if m % 2 == 1:
                grp, kind = BC_SLOTS[m // 2]
                bcast_group(1, grp, kind, False)
            if pending_gate is not None:
                pending_gate()
            pending_gate = g
        pending_gate()
        # keep the ACT/DVE queues fed before chunk-0 out_proj drains
        scan_m(1, 0)
        wout16 = load_wout()
        for lt in range(4):
            out_proj_lt(0, lt, binb0, lt, wout16)
        nc.gpsimd.collective_compute(
            "ReduceScatter", OP.add, replica_groups=REPLICA_GROUPS,
            ins=[binb0.opt()], outs=[bout0.opt()],
        )
        for m in range(1, 4):
            scan_m(1, m)
            if m % 2 == 1:
                grp, kind = BC_SLOTS[m // 2]
                bcast_group(2, grp, kind, False)
        ln_tile(bout0, 0, 0)
        for m in range(4, MH):
            scan_m(1, m)
            if m % 2 == 1:
                grp, kind = BC_SLOTS[m // 2]
                bcast_group(2, grp, kind, False)
        ln_tile(bout0, 1, 1)
        scan_m(2, 0)
        for lt in range(2):
            out_proj_lt(1, lt, binbq[0], lt, wout16)
        nc.gpsimd.collective_compute(
            "ReduceScatter", OP.add, replica_groups=REPLICA_GROUPS,
            ins=[binbq[0].opt()], outs=[boutq[0].opt()],
        )
        for m in range(1, 4):
            scan_m(2, m)
        ln_tile(boutq[0], 0, 2)
        for m in range(4, MH):
            scan_m(2, m)
        for lt in range(2):
            out_proj_lt(2, lt, binbq[1], lt, wout16)
        nc.gpsimd.collective_compute(
            "ReduceScatter", OP.add, replica_groups=REPLICA_GROUPS,
            ins=[binbq[1].opt()], outs=[boutq[1].opt()],
        )
        ln_tile(boutq[1], 0, 3)

    nc.compile()
    return nc


def _shard_inputs(inputs):
    x = np.asarray(inputs["x"], np.float32)
    W_in = np.asarray(inputs["W_in"], np.float32)
    conv_w = np.asarray(inputs["conv_w"], np.float32)
    conv_b = np.asarray(inputs["conv_b"], np.float32)
    W_x = np.asarray(inputs["W_x"], np.float32)
    W_dt = np.asarray(inputs["W_dt"], np.float32)
    b_dt = np.asarray(inputs["b_dt"], np.float32)
    Dp = np.asarray(inputs["D"], np.float32)
    W_out = np.asarray(inputs["W_out"], np.float32)
    gamma = np.asarray(inputs["gamma"], np.float32)
    beta = np.asarray(inputs["beta"], np.float32)

    def col_tiles(v):  # [DH] -> [P, MH] (tile-major columns)
        return np.ascontiguousarray(v.reshape(-1, P).T)

    FP8NP = ml_dtypes.float8_e4m3fn
    BF16NP = ml_dtypes.bfloat16

    in_maps = []
    for c in range(N_CORES):
        b, half = divmod(c, 2)
        own = np.arange(half * DH, (half + 1) * DH)
        cw = conv_w[own]  # [1024, 4]
        cdiag = np.zeros((P, MH * D_CONV * P), np.float32)
        for mt in range(MH):
            for j in range(D_CONV):
                blk = cdiag[:, (mt * D_CONV + j) * P : (mt * D_CONV + j + 1) * P]
                np.fill_diagonal(blk, cw[mt * P : (mt + 1) * P, j] * CSCALE)
        dvd = np.zeros((P, MH * P), np.float32)
        dloc = Dp[half * DH : (half + 1) * DH]
        for m in range(MH):
            np.fill_diagonal(dvd[:, m * P : (m + 1) * P], dloc[m * P : (m + 1) * P])
        rows = [x[b, half * LQ : (half + 1) * LQ],
                x[b, 512 + half * P : 512 + half * P + P],
                x[b, 768 + half * P : 768 + half * P + P]]
        m = {
            "xt": np.ascontiguousarray(x[b].T).astype(FP8NP),
            "xres": np.ascontiguousarray(np.concatenate(rows, axis=0)).astype(BF16NP),
            "win": np.ascontiguousarray(
                WSCALE
                * np.concatenate(
                    [W_in[:, half * DH : (half + 1) * DH],
                     W_in[:, D_INNER + half * DH : D_INNER + (half + 1) * DH]],
                    axis=1,
                )
            ).astype(FP8NP),
            "convdiag": cdiag.astype(FP8NP),
            "convb": np.ascontiguousarray(conv_b[own].reshape(MH, P).T),
            "wx": np.ascontiguousarray(W_x[own]).astype(BF16NP),
            "wdt": np.ascontiguousarray(W_dt[:, half * DH : (half + 1) * DH]).astype(BF16NP),
            "bdt": col_tiles(b_dt[half * DH : (half + 1) * DH]),
            "dvdiag": dvd.astype(BF16NP),
            "wout": np.ascontiguousarray(W_out[half * DH : (half + 1) * DH]).astype(BF16NP),
            "gamma": np.ascontiguousarray(gamma[None, :]).astype(BF16NP),
            "beta": np.ascontiguousarray(beta[None, :]).astype(BF16NP),
            "eye": np.eye(P, dtype=np.float32).astype(BF16NP),
        }
        in_maps.append(m)
    return in_maps


def derive_a_n(inputs):
    A_log = np.asarray(inputs["A_log"], np.float32)
    return tuple(float(v) for v in (-np.exp(A_log[0, :])))


_PROGRAM_CACHE = {}


def get_program(a_n):
    key = a_n
    if key not in _PROGRAM_CACHE:
        _PROGRAM_CACHE[key] = build_program(a_n)
    return _PROGRAM_CACHE[key]


def assemble(results):
    out = np.empty((BATCH, SEQ, D_MODEL), np.float32)
    for c in range(N_CORES):
        b, half = divmod(c, 2)
        r = results[c]["out_half"]
        out[b, half * LQ : (half + 1) * LQ] = r[0:LQ]
        out[b, 512 + half * P : 512 + half * P + P] = r[LQ : LQ + P]
        out[b, 768 + half * P : 768 + half * P + P] = r[LQ + P : LQ + 2 * P]
    return out


def kernel(**inputs):
    from concourse import bass_utils

    a_n = derive_a_n(inputs)
    nc = get_program(a_n)
    in_maps = _shard_inputs(inputs)
    res = bass_utils.run_bass_kernel_spmd(nc, in_maps, core_ids=list(range(N_CORES)))
    return assemble(res.results)
